# revision 1
# baseline (speedup 1.0000x reference)
"""Grouped (MoE-routed) GEMM on 8 Trainium2 NeuronCores.

out[m, n] = sum_k lhs[m, k] * rhs[g[m], n, k],  g = clamp(m_indices, 0, G)

Strategy: expert-parallel. Host dispatches rows by m_indices (the
"all-to-all" is a host-side gather since we hold full inputs), core c gets
expert c's rows padded to a common M_pad, plus expert c's weight matrix.
Every core then runs one identical dense GEMM program:

    o[M_pad, N] = A[M_pad, K] @ B[N, K]^T     (bf16 in, fp32 accum, bf16 out)

Both operands are pre-transposed to K-major on the host (shard-prep layout),
so the device only issues full-rate contiguous DMAs — no on-device
transposes. PE streams 128x512 matmuls accumulating over 8 K-chunks in PSUM;
DVE casts PSUM->SBUF bf16; ACT-ring DMAs store to DRAM.
"""

import numpy as np
import ml_dtypes

K = 1024
N = 2048
G = 8
N_CORES = 8
KP = 128           # SBUF partitions / contraction chunk
KC = K // KP       # 8 k-chunks
NB = 512           # psum free-dim chunk (one PSUM bank of fp32)

_BUILD_CACHE = {}


def _build(m_pad):
    import concourse.mybir as mybir
    import concourse.tile as tile
    from concourse import bacc

    if m_pad in _BUILD_CACHE:
        return _BUILD_CACHE[m_pad]

    nc = bacc.Bacc("TRN2", target_bir_lowering=False, debug=False,
                   num_devices=N_CORES)

    at_d = nc.dram_tensor("at", [KC, KP, m_pad], mybir.dt.bfloat16,
                          kind="ExternalInput")
    bt_d = nc.dram_tensor("bt", [KC, KP, N], mybir.dt.bfloat16,
                          kind="ExternalInput")
    o_d = nc.dram_tensor("o", [m_pad, N], mybir.dt.bfloat16,
                         kind="ExternalOutput")

    mt_n = m_pad // KP
    nb_n = N // NB
    MC = 512  # granule of A-transpose load along m

    with tile.TileContext(nc) as tc:
        with (
            tc.tile_pool(name="ats", bufs=1) as ap,
            tc.tile_pool(name="bts", bufs=1) as bp,
            tc.tile_pool(name="ost", bufs=4) as op,
            tc.tile_pool(name="ps", bufs=8, space="PSUM") as pp,
        ):
            at_s = ap.tile([KP, KC, m_pad], mybir.dt.bfloat16)
            bt_s = bp.tile([KP, KC, N], mybir.dt.bfloat16)

            # Loads, ordered so the first psum tile's deps land first:
            # B column 0, then A by m-chunks, then remaining B columns.
            for kc in range(KC):
                nc.sync.dma_start(bt_s[:, kc, 0:NB], bt_d[kc, :, 0:NB])
            for mc in range(0, m_pad, MC):
                mw = min(MC, m_pad - mc)
                for kc in range(KC):
                    nc.sync.dma_start(at_s[:, kc, mc:mc + mw],
                                      at_d[kc, :, mc:mc + mw])
            for nb in range(1, nb_n):
                for kc in range(KC):
                    nc.sync.dma_start(bt_s[:, kc, nb * NB:(nb + 1) * NB],
                                      bt_d[kc, :, nb * NB:(nb + 1) * NB])

            # GEMM sweep: for each output column block, walk all m-tiles.
            for nb in range(nb_n):
                for mt in range(mt_n):
                    p = pp.tile([KP, NB], mybir.dt.float32)
                    for kc in range(KC):
                        nc.tensor.matmul(
                            p[:],
                            at_s[:, kc, mt * KP:(mt + 1) * KP],
                            bt_s[:, kc, nb * NB:(nb + 1) * NB],
                            start=(kc == 0),
                            stop=(kc == KC - 1),
                        )
                    ot = op.tile([KP, NB], mybir.dt.bfloat16)
                    nc.vector.tensor_copy(ot[:], p[:])
                    nc.scalar.dma_start(
                        o_d[mt * KP:(mt + 1) * KP, nb * NB:(nb + 1) * NB],
                        ot[:])

    nc.compile()
    _BUILD_CACHE[m_pad] = nc
    return nc


def kernel(lhs, rhs, m_indices):
    from concourse import bass_utils

    lhs = np.asarray(lhs)
    rhs = np.asarray(rhs)
    m_indices = np.asarray(m_indices)
    M = lhs.shape[0]

    g = np.where((m_indices >= 0) & (m_indices < G), m_indices, 0)
    rows = [np.nonzero(g == e)[0] for e in range(G)]
    m_max = max(len(r) for r in rows)
    m_pad = max(((m_max + KP - 1) // KP) * KP, KP)

    nc = _build(m_pad)

    in_maps = []
    for e in range(G):
        a = np.zeros((m_pad, K), dtype=ml_dtypes.bfloat16)
        a[:len(rows[e])] = lhs[rows[e]]
        at = a.T.reshape(KC, KP, m_pad)          # [k, m] -> [kc, kp, m]
        bt = rhs[e].T.reshape(KC, KP, N)         # [n, k] -> [kc, kp, n]
        in_maps.append({
            "at": np.ascontiguousarray(at),
            "bt": np.ascontiguousarray(bt),
        })

    res = bass_utils.run_bass_kernel_spmd(nc, in_maps,
                                          core_ids=list(range(N_CORES)))

    out = np.zeros((M, N), dtype=ml_dtypes.bfloat16)
    for e in range(G):
        out[rows[e]] = res.results[e]["o"][:len(rows[e])]
    return out


# revision 4
# speedup vs baseline: 1.0016x; 1.0016x over previous
"""Grouped (MoE-routed) GEMM on 8 Trainium2 NeuronCores.

out[m, n] = sum_k lhs[m, k] * rhs[g[m], n, k],  g = clamp(m_indices, 0, G)

Strategy: expert-parallel. Host dispatches rows by m_indices (the
"all-to-all" is a host-side gather since we hold full inputs), core c gets
expert c's rows padded to a common M_pad, plus expert c's weight matrix.
Every core then runs one identical dense GEMM program computing the
transposed output:

    oT[N, M_pad] = B[N, K] @ A[M_pad, K]^T    (bf16 in, fp32 accum, bf16 out)

Both operands are pre-transposed to K-major on the host (shard-prep
layout), so the device only issues full-rate contiguous DMAs — no
on-device transposes. B tiles are the stationary operand (one LDWEIGHTS
per [128k x 128n] tile, amortized over the whole m sweep), A is the moving
operand so the padded tail m-chunk can be 64-wide instead of a full 128
m-tile. PE streams 512-row matmuls accumulating over 8 K-chunks in PSUM;
DVE casts PSUM->SBUF bf16; the ACT HWDGE ring stores oT to DRAM while the
SP ring carries B loads and ACT carries A loads.
"""

import numpy as np
import ml_dtypes

K = 1024
N = 2048
G = 8
N_CORES = 8
KP = 128           # SBUF partitions / contraction chunk
KC = K // KP       # 8 k-chunks
MCH = 512          # moving-operand m-chunk (one PSUM bank of fp32)

_BUILD_CACHE = {}


def _m_chunks(m_pad):
    """Split m_pad into moving-operand chunks of <=512, 64-aligned."""
    chunks = []
    m = 0
    while m < m_pad:
        w = min(MCH, m_pad - m)
        chunks.append((m, w))
        m += w
    return chunks


def _build(m_pad):
    import concourse.mybir as mybir
    import concourse.tile as tile
    from concourse import bacc

    if m_pad in _BUILD_CACHE:
        return _BUILD_CACHE[m_pad]

    nc = bacc.Bacc("TRN2", target_bir_lowering=False, debug=False,
                   num_devices=N_CORES)

    at_d = nc.dram_tensor("at", [KC, KP, m_pad], mybir.dt.bfloat16,
                          kind="ExternalInput")
    bt_d = nc.dram_tensor("bt", [KC, KP, N], mybir.dt.bfloat16,
                          kind="ExternalInput")
    o_d = nc.dram_tensor("o", [N, m_pad], mybir.dt.bfloat16,
                         kind="ExternalOutput")

    nt_n = N // KP           # 16 stationary n-tiles
    chunks = _m_chunks(m_pad)

    with tile.TileContext(nc) as tc:
        with (
            tc.tile_pool(name="ats", bufs=1) as ap,
            tc.tile_pool(name="bts", bufs=1) as bp,
            tc.tile_pool(name="ost", bufs=4) as op,
            tc.tile_pool(name="ps", bufs=8, space="PSUM") as pp,
        ):
            at_s = ap.tile([KP, KC, m_pad], mybir.dt.bfloat16)
            bt_s = bp.tile([KP, KC, N], mybir.dt.bfloat16)

            # Coarse loads: one DMA per k-chunk per operand, B on the SP
            # ring, A on the ACT ring, so issue cost runs in parallel and
            # the first (bt[0], at[0]) pair lands ~2us after kernel start.
            for kc in range(KC):
                nc.sync.dma_start(bt_s[:, kc, :], bt_d[kc])
                nc.scalar.dma_start(at_s[:, kc, :], at_d[kc])

            # GEMM sweep, kc-outer per n-tile: stationary bt tile is
            # reloaded once per (nt, kc) and swept over every m-chunk.
            for nt in range(nt_n):
                ps = [pp.tile([KP, w], mybir.dt.float32, name=f"ps{ci}",
                              tag="ps")
                      for ci, (_, w) in enumerate(chunks)]
                for kc in range(KC):
                    b_tile = bt_s[:, kc, nt * KP:(nt + 1) * KP]
                    for ci, (mc, w) in enumerate(chunks):
                        nc.tensor.matmul(
                            ps[ci][:],
                            b_tile,
                            at_s[:, kc, mc:mc + w],
                            start=(kc == 0),
                            stop=(kc == KC - 1),
                        )
                for ci, (mc, w) in enumerate(chunks):
                    ot = op.tile([KP, w], mybir.dt.bfloat16)
                    nc.vector.tensor_copy(ot[:], ps[ci][:])
                    nc.scalar.dma_start(
                        o_d[nt * KP:(nt + 1) * KP, mc:mc + w], ot[:])

    nc.compile()
    _BUILD_CACHE[m_pad] = nc
    return nc


def _prep_in_maps(lhs, rhs, rows, m_pad):
    in_maps = []
    for e in range(G):
        a = np.zeros((m_pad, K), dtype=ml_dtypes.bfloat16)
        a[:len(rows[e])] = lhs[rows[e]]
        at = a.T.reshape(KC, KP, m_pad)          # [k, m] -> [kc, kp, m]
        bt = rhs[e].T.reshape(KC, KP, N)         # [n, k] -> [kc, kp, n]
        in_maps.append({
            "at": np.ascontiguousarray(at),
            "bt": np.ascontiguousarray(bt),
        })
    return in_maps


def _shard(m_indices):
    g = np.where((m_indices >= 0) & (m_indices < G), m_indices, 0)
    rows = [np.nonzero(g == e)[0] for e in range(G)]
    m_max = max(len(r) for r in rows)
    m_pad = max(-(-m_max // 64) * 64, 128)
    return rows, m_pad


def kernel(lhs, rhs, m_indices):
    from concourse import bass_utils

    lhs = np.asarray(lhs)
    rhs = np.asarray(rhs)
    m_indices = np.asarray(m_indices)
    M = lhs.shape[0]

    rows, m_pad = _shard(m_indices)
    nc = _build(m_pad)
    in_maps = _prep_in_maps(lhs, rhs, rows, m_pad)

    res = bass_utils.run_bass_kernel_spmd(nc, in_maps,
                                          core_ids=list(range(N_CORES)))

    out = np.zeros((M, N), dtype=ml_dtypes.bfloat16)
    for e in range(G):
        oT = res.results[e]["o"]                 # [N, m_pad]
        out[rows[e]] = oT[:, :len(rows[e])].T
    return out
